# revision 9
# baseline (speedup 1.0000x reference)
"""Bass/Trainium2 kernel for nn_CustomLoss_87952340287807.

Loss over B=8,388,608 Euler-angle triples:
    per-sample = 1 - |cross(vo, vt)| + dot(vo, vt),  summed.
vo/vt are unit vectors, so |cross| = sqrt(1 - dot^2) and only dot is needed:
  dot = cosD*(u*U + v*V) + sinD*(u*V - v*U) + w*W
  u = sin(P)cos(R), v = sin(R), w = cos(P)cos(R)   (o side; caps = t side)
  D = 2*pi*(yt - yo)
Inputs are pre-shifted on the host (t = x - 0.5) so every angle is 2*pi*t,
t in [-0.5, 0.5]: full sines and half-angle sines come straight off the
ScalarE Sin LUT with zero bias, all in-domain. Cosines are folded into
fused custom-DVE ops on the half-sines:
  COSMUL:  (1 - 2*h^2) * y        (u = cos(R)*sin(P), cosD*a)
  COSPROD: (1-2*a^2)(1-2*b^2)     (cos*cos products for w_o*w_t)
  SINMUL:  (2 - 4*j^2) * y        (sinD*b via hD*(...))
Both sides' sines land in single tiles (sff = [sp_o|sr_o|sp_t|sr_t],
hh likewise), so the u/U COSMUL and x1/x2 COSPROD run as one 2-page
instruction each (same cycles, half the instruction overhead).
Engine balance (ScalarE 0.83 ns/elem, DVE custom 1.04, DVE stock f16
0.52): ScalarE evaluates 10 LUT elems/sample + the dot^2 Square (one
segment delayed so it never stalls the sine stream) + deferred Sqrt
passes; DVE runs the customs + the stock multiply chain; PE reduces
sum(dot) via ones-matmuls into PSUM; GPSIMD only triggers DMA (its
compute path steals DVE SBUF ports).

Sharding: pure data-parallel, batch split across 8 NeuronCores; each core
returns 3 per-partition cross-norm sums + a [1,512] PSUM dot sum; host
reduces.
"""
import sys

import numpy as np

if "/opt/trn_rl_repo" not in sys.path:
    sys.path.insert(0, "/opt/trn_rl_repo")

import concourse.bacc as bacc
import concourse.mybir as mybir
import concourse.tile as tile
from concourse import dve_ops as dvo
from concourse.bass_utils import run_bass_kernel_spmd
from concourse.dve_spec import One, Spec, Src0, Src1, _has_src1, lower
from concourse.dve_spec import C0, C1, sq
from concourse.dve_uop import DveOpSpec

B = 8388608
NCORES = 8
S = B // NCORES          # 1,048,576 samples per core
P = 128
SP = S // P              # 8192 samples per partition
F = 1344                 # samples per partition per full segment
SEGS = [(0, 256), (256, 512), (768, 1024),
        (1792, 1344), (3136, 1344), (4480, 1344), (5824, 1344),
        (7168, 512), (7680, 256), (7936, 256)]
NSEG = len(SEGS)
MMW = 256                # psum accumulator width for the dot reduction

AF = mybir.ActivationFunctionType
ALU = mybir.AluOpType
dt = mybir.dt
f32, f16 = dt.float32, dt.float16
PI = float(np.pi)

_cache = {}
last_results = None


def _reg(name, spec):
    """Register a custom DVE op at runtime (per-NEFF table, no firmware
    change). Computes the pinned uops sha the same way DveOp.compile does."""
    for op in dvo.OPS:
        if op.name == name:
            return op
    row = dvo._CUSTOM_DVE_ROW_BASE + len(dvo.OPS)
    assert row < 0x20, "custom-DVE opcode rows exhausted"
    ver = "v3"  # TRN2
    uops = lower(spec, ver=ver)
    sha = DveOpSpec(name=name, opcode=row, uops=uops,
                    rd1_en=_has_src1(spec)).sha(ver)
    op = dvo.DveOp(name, spec, subdim=False, uops_sha={ver: sha})
    dvo.OPS.append(op)
    dvo._SUB_OPCODE_FOR_NAME[name] = row
    dvo.CUSTOM_DVE_SPECS[name] = spec
    return op


# (1-2*Src0^2) * (1-2*Src1^2) = cosA*cosB from half-sines (s0=2)
COSPROD = _reg("COSPROD_ANT", Spec(
    body=(One - sq(Src0) * C0) * (One - sq(Src1) * C0)))
# (1 - s0*h^2) * y; s0=2: cosA * y from half-sine
COSMUL = _reg("COSMUL_ANT", Spec(
    body=(One - sq(Src0) * C0) * Src1))
# (s0 - s1*j^2) * y; s0=2, s1=4: (sinD/hD) * y
SINMUL = _reg("SINMUL_ANT", Spec(
    body=(C0 - sq(Src0) * C1) * Src1))


def _build():
    nc = bacc.Bacc("TRN2", target_bir_lowering=False, debug=False)
    o_in = nc.declare_dram_parameter("out_in", [S, 3], f32, isOutput=False)
    t_in = nc.declare_dram_parameter("tgt_in", [S, 3], f32, isOutput=False)
    res = nc.declare_dram_parameter("res", [P, 5], f32, isOutput=True)
    res2 = nc.declare_dram_parameter("res2", [1, MMW], f32, isOutput=True)

    o_flat = o_in.ap().rearrange("(p n) c -> p (n c)", p=P)
    t_flat = t_in.ap().rearrange("(p n) c -> p (n c)", p=P)

    with tile.TileContext(nc) as tc:
        with tc.tile_pool(name="consts", bufs=1) as cpool, \
             tc.tile_pool(name="raw", bufs=3) as rawpool, \
             tc.tile_pool(name="sb", bufs=1) as pool, \
             tc.tile_pool(name="psum", bufs=1, space="PSUM") as psum, \
             tc.tile_pool(name="persist", bufs=1) as ppool:
            one_b = cpool.tile([P, 1], f32, name="one_b", tag="one_b")
            nc.vector.memset(one_b[:], 1.0)
            ones = cpool.tile([P, 1], f16, name="ones", tag="ones")
            nc.vector.memset(ones[:], 1.0)

            q_all = ppool.tile([P, SP], f16, name="q_all", tag="q_all")
            accs = ppool.tile([P, 5], f32, name="accs", tag="accs")
            dps = psum.tile([1, MMW], f32, name="dps", tag="dps")

            def mk(tag, w, full=F, bufs=None):
                t = pool.tile([P, full], f16, name=tag, tag=tag, bufs=bufs)
                return t[:, :w]

            def load(col0, fw):
                ro = rawpool.tile([P, 3 * F], f16, name="raw_o", tag="raw_o")
                nc.gpsimd.dma_start(ro[:, :3 * fw],
                                    o_flat[:, 3 * col0:3 * (col0 + fw)])
                rt = rawpool.tile([P, 3 * F], f16, name="raw_t", tag="raw_t")
                nc.gpsimd.dma_start(rt[:, :3 * fw],
                                    t_flat[:, 3 * col0:3 * (col0 + fw)])
                return ro, rt

            n_mm = sum((fw + MMW - 1) // MMW for _, fw in SEGS)
            mm_i = 0
            prev_sq = None   # (hc_tile, col0, fw) squared one segment late

            pending = [load(*SEGS[0]), load(*SEGS[1])]
            for i, (col0, fw) in enumerate(SEGS):
                raw_o, raw_t = pending.pop(0)

                ov = raw_o[:, :3 * fw].rearrange("p (n c) -> p c n", c=3)
                tv = raw_t[:, :3 * fw].rearrange("p (n c) -> p c n", c=3)
                yo, yt = ov[:, 0, :], tv[:, 0, :]
                pr_o, pr_t = ov[:, 1:3, :], tv[:, 1:3, :]

                # yaw delta first so hD/jD can issue right after the sines
                e = mk("e", fw, bufs=2)
                nc.vector.tensor_sub(e, yt, yo)

                # ScalarE LUT block into single tiles:
                # sff = [sp_o | sr_o | sp_t | sr_t], hh = [hp_o | hr_o | hp_t | hr_t]
                sff = mk("sff", 4 * fw, 4 * F, bufs=2)
                nc.scalar.activation(
                    sff[:, :2 * fw].rearrange("p (c n) -> p c n", c=2),
                    pr_o, AF.Sin, scale=2 * PI)
                nc.scalar.activation(
                    sff[:, 2 * fw:4 * fw].rearrange("p (c n) -> p c n", c=2),
                    pr_t, AF.Sin, scale=2 * PI)
                hh = mk("hh", 4 * fw, 4 * F, bufs=2)
                nc.scalar.activation(
                    hh[:, :2 * fw].rearrange("p (c n) -> p c n", c=2),
                    pr_o, AF.Sin, scale=PI)
                nc.scalar.activation(
                    hh[:, 2 * fw:4 * fw].rearrange("p (c n) -> p c n", c=2),
                    pr_t, AF.Sin, scale=PI)
                hD = mk("hD", fw, bufs=2)
                nc.scalar.activation(hD, e, AF.Sin, scale=PI)
                jD = mk("jD", fw, bufs=2)
                nc.scalar.activation(jD, e, AF.Sin, scale=PI / 2)
                # square previous segment's clamped dot (delayed emission so
                # ScalarE never stalls this segment's LUT stream)
                if prev_sq is not None:
                    hc_p, c0p, fwp = prev_sq
                    nc.scalar.activation(q_all[:, c0p:c0p + fwp], hc_p,
                                         AF.Square)
                    prev_sq = None

                # prefetch two segments ahead
                if i + 2 < NSEG:
                    pending.append(load(*SEGS[i + 2]))

                sp_o, sp_t = sff[:, :fw], sff[:, 2 * fw:3 * fw]
                sr_o, sr_t = sff[:, fw:2 * fw], sff[:, 3 * fw:4 * fw]
                hp_o, hp_t = hh[:, :fw], hh[:, 2 * fw:3 * fw]
                hr_o, hr_t = hh[:, fw:2 * fw], hh[:, 3 * fw:4 * fw]

                # DVE: fused cosine customs
                u = mk("u", fw)
                nc.vector._custom_dve(COSMUL, out=u, in0=hr_o, in1=sp_o,
                                      s0=2.0)
                U_ = mk("U_", fw)
                nc.vector._custom_dve(COSMUL, out=U_, in0=hr_t, in1=sp_t,
                                      s0=2.0)
                x1 = mk("x1", fw)
                nc.vector._custom_dve(COSPROD, out=x1, in0=hp_o, in1=hp_t,
                                      s0=2.0)
                x2 = mk("x2", fw)
                nc.vector._custom_dve(COSPROD, out=x2, in0=hr_o, in1=hr_t,
                                      s0=2.0)
                g = mk("g", fw)
                nc.vector.tensor_mul(g, x1, x2)
                m1 = mk("m1", fw)
                nc.vector.tensor_mul(m1, u, U_)
                m2 = mk("m2", fw)
                nc.vector.tensor_mul(m2, sr_o, sr_t)
                a = mk("a", fw)
                nc.vector.tensor_add(a, m1, m2)
                m3 = mk("m3", fw)
                nc.vector.tensor_mul(m3, u, sr_t)
                m4 = mk("m4", fw)
                nc.vector.tensor_mul(m4, sr_o, U_)
                b = mk("b", fw)
                nc.vector.tensor_sub(b, m3, m4)

                # rotation by D: p1 = cosD*a, q1 = sinD*b (fused)
                p1 = mk("m1", fw)
                nc.vector._custom_dve(COSMUL, out=p1, in0=hD, in1=a, s0=2.0)
                t2b = mk("m3", fw)
                nc.vector._custom_dve(SINMUL, out=t2b, in0=jD, in1=b,
                                      s0=2.0, s1=4.0)
                q1 = mk("m4", fw)
                nc.vector.tensor_mul(q1, hD, t2b)
                s = mk("m2", fw)
                nc.vector.tensor_add(s, p1, q1)
                dot = mk("dot", fw, bufs=2)
                nc.vector.tensor_add(dot, g, s)

                # sum(dot) on the idle PE: ones[128,1]^T @ dot -> [1, MMW]
                for c0 in range(0, fw, MMW):
                    cw = min(MMW, fw - c0)
                    nc.tensor.matmul(dps[:, :cw], ones[:], dot[:, c0:c0 + cw],
                                     start=(mm_i == 0), stop=(mm_i == n_mm - 1))
                    mm_i += 1

                # clamp; Square lands on ScalarE next segment
                hc = mk("hc", fw, bufs=2)
                nc.vector.tensor_scalar(hc, dot, -1.0, 1.0, ALU.max, ALU.min)
                prev_sq = (hc, col0, fw)

            # last segment's square, then deferred sqrt passes (outputs go
            # into the by-then-dead raw buffers; only accum matters)
            hc_p, c0p, fwp = prev_sq
            nc.scalar.activation(q_all[:, c0p:c0p + fwp], hc_p, AF.Square)
            CUTS = [0, 3136, 5824, 7680, 8192]
            for k in range(4):
                lo, hi = CUTS[k], CUTS[k + 1]
                dump = rawpool.tile([P, 3 * F], f16, name=f"cn{k}",
                                    tag=("raw_o" if k % 2 == 0 else "raw_t"))
                nc.scalar.activation(dump[:, :hi - lo], q_all[:, lo:hi],
                                     AF.Sqrt, bias=one_b[:], scale=-1.0,
                                     accum_out=accs[:, k:k + 1])
            nc.vector.memset(accs[:, 4:5], 0.0)

            dsb = ppool.tile([1, MMW], f32, name="dsb", tag="dsb")
            nc.vector.tensor_copy(dsb[:], dps[:])

            nc.sync.dma_start(res[:], accs[:])
            nc.sync.dma_start(res2[:], dsb[:])

    nc.compile()
    return nc


def kernel(output: np.ndarray, target: np.ndarray) -> np.ndarray:
    global last_results
    if "nc" not in _cache:
        _cache["nc"] = _build()
    nc = _cache["nc"]

    # host-side pre-shift: angles become 2*pi*t with t in [-0.5, 0.5]
    output = np.ascontiguousarray(output, dtype=np.float32) - np.float32(0.5)
    target = np.ascontiguousarray(target, dtype=np.float32) - np.float32(0.5)
    in_maps = [
        {"out_in": output[c * S:(c + 1) * S], "tgt_in": target[c * S:(c + 1) * S]}
        for c in range(NCORES)
    ]
    r = run_bass_kernel_spmd(nc, in_maps, list(range(NCORES)))
    last_results = r

    total = np.float64(B)
    for c in range(NCORES):
        out = r.results[c]["res"].astype(np.float64)
        out2 = r.results[c]["res2"].astype(np.float64)
        total += out2.sum() - out[:, 0:4].sum()
    return np.float32(total)
